# revision 3
# baseline (speedup 1.0000x reference)
"""Trainium2 Bass kernel: 16-head causal attention with sink logit.

Contract: kernel(**inputs) takes the FULL inputs of the reference
(x [2,2048,1024], W_Q/W_K/W_V/W_out [1024,1024], sink [16]) and returns
the FULL output [2,2048,1024], running on 8 NeuronCores.

Sharding: core c = b*4 + g handles batch b and heads [4g, 4g+4).
Each core computes yT_partial [1024, 2048] = W_out_slice^T @ attn^T;
host sums the 4 partials per batch and transposes.

v2 design (vs baseline):
- scores in bf16: FWL weight loads + 2-way row-tiled (K=64) head pairs
  run concurrently on row-groups 0/64 of the PE array.
- PV in bf16 with the ones-row denominator trick (V-stationary, output
  is attn^T directly, M=65/128 padded lhsT).
- exp batched [128,1024] straight from PSUM on ScalarE.
- q-blocks of 256 (less masked-region waste); f32r projections chopped
  into 256-token chunks woven between attention steps so the PE always
  has work while ScalarE runs the exp stream.
"""

import sys
import numpy as np

if "/opt/trn_rl_repo" not in sys.path:
    sys.path.insert(0, "/opt/trn_rl_repo")

B, T, C = 2, 2048, 1024
H, D = 16, 64
G = 4                # heads per core
DH = G * D           # 256 head-dims per core
NCORES = 8
QB = 256             # q block (attention step granularity)
NQB = T // QB        # 8
NKT = T // 128       # 16 k-tiles
NCC = C // 128       # 8 contraction chunks over C
SCALE = 1.0 / float(np.sqrt(D))

# vp per-kt slot layout (386 cols per kt), bf16:
#   pair p head A: [V(64) | one]             off p*193+0,  width 65,  denom row 64
#   pair p head B: [one | zeros(63) | V(64)] off p*193+65, width 128, denom row 0
VP_W = 386
VP_OFF = [0, 65, 193, 258]


def build_program(reps=1):
    """Build the per-core Bass program. reps>1 repeats the compute body
    (same inputs -> same outputs) for differential wall-clock timing."""
    from contextlib import ExitStack

    import concourse.bass as bass
    import concourse.tile as tile
    from concourse import bacc, mybir

    f32 = mybir.dt.float32
    f32r = mybir.dt.float32r
    bf16 = mybir.dt.bfloat16
    AF = mybir.ActivationFunctionType
    Alu = mybir.AluOpType

    nc = bacc.Bacc("TRN2", target_bir_lowering=False, debug=False)

    xt_d = nc.dram_tensor("xt", [C, T], f32r, kind="ExternalInput").ap()
    wq_d = nc.dram_tensor("wq", [C, DH], f32r, kind="ExternalInput").ap()
    wk_d = nc.dram_tensor("wk", [C, DH], f32r, kind="ExternalInput").ap()
    wv_d = nc.dram_tensor("wv", [C, DH], f32r, kind="ExternalInput").ap()
    wo_d = nc.dram_tensor("wo", [DH, C], bf16, kind="ExternalInput").ap()
    sk_d = nc.dram_tensor("sk", [1, G], f32, kind="ExternalInput").ap()
    cm_d = nc.dram_tensor("cm", [128, 1024], bf16, kind="ExternalInput").ap()
    vpc_d = nc.dram_tensor("vpc", [128, NKT * 65], bf16, kind="ExternalInput").ap()
    ind_d = nc.dram_tensor("ind", [128, 128], f32r, kind="ExternalInput").ap()
    yt_d = nc.dram_tensor("yt", [C, T], f32, kind="ExternalOutput").ap()

    xt_v = xt_d.rearrange("(n p) m -> p n m", p=128)   # [128, 8, 2048]
    wq_v = wq_d.rearrange("(n p) m -> p n m", p=128)   # [128, 8, 256]
    wk_v = wk_d.rearrange("(n p) m -> p n m", p=128)
    wv_v = wv_d.rearrange("(n p) m -> p n m", p=128)
    wo_v = wo_d.rearrange("(n p) m -> p n m", p=128)   # [128, 2, 1024]
    yt_v = yt_d.rearrange("(n p) m -> p n m", p=128)   # [128, 8, 2048]

    with tile.TileContext(nc) as tc, ExitStack() as ctx:
        P = lambda name, bufs: ctx.enter_context(tc.tile_pool(name=name, bufs=bufs))
        const_p = P("const", 1)
        big_p = P("big", 1)
        y_p = P("y", 2)
        row_p = P("row", 2)
        # PSUM: ps_s 2x2 banks + ps_o 2x1 + ps_w 2x1 = 8 banks
        ps_s = ctx.enter_context(tc.tile_pool(name="ps_s", bufs=2, space="PSUM"))
        ps_o = ctx.enter_context(tc.tile_pool(name="ps_o", bufs=2, space="PSUM"))
        ps_w = ctx.enter_context(tc.tile_pool(name="ps_w", bufs=2, space="PSUM"))

        # ---- persistent SBUF tensors ----
        xt_sb = big_p.tile([128, NCC * T], f32r, tag="xt")           # 64KB/part
        wq_sb = big_p.tile([128, NCC * DH], f32r, tag="wq")
        wk_sb = big_p.tile([128, NCC * DH], f32r, tag="wk")
        wv_sb = big_p.tile([128, NCC * DH], f32r, tag="wv")
        wo_sb = big_p.tile([128, 2 * C], bf16, tag="wo")
        qt_sb = big_p.tile([128, 2 * T], bf16, tag="qt")
        kt_sb = big_p.tile([128, 2 * T], bf16, tag="kt")
        vp_sb = big_p.tile([128, NKT * VP_W], bf16, tag="vp")
        pab_sb = big_p.tile([128, 3 * NKT * 512], bf16, tag="pab")   # ring of 3
        at_sb = big_p.tile([128, 2 * T], bf16, tag="at")             # attn^T normalized
        cm_sb = const_p.tile([128, 1024], bf16, tag="cm")
        ind_sb = const_p.tile([128, 128], f32r, tag="ind")
        skr_sb = const_p.tile([128, G], f32, tag="skr")
        esk_sb = const_p.tile([128, G], f32, tag="esk")

        # ---- one-time loads + constants (outside reps) ----
        for i in range(NCC):
            nc.sync.dma_start(xt_sb[:, i * T:(i + 1) * T], xt_v[:, i, :])
        nc.sync.dma_start(
            wq_sb[:].rearrange("p (n m) -> p n m", m=DH), wq_v[:, :, :])
        nc.sync.dma_start(
            wk_sb[:].rearrange("p (n m) -> p n m", m=DH), wk_v[:, :, :])
        nc.sync.dma_start(
            wv_sb[:].rearrange("p (n m) -> p n m", m=DH), wv_v[:, :, :])
        nc.sync.dma_start(
            wo_sb[:].rearrange("p (n m) -> p n m", m=C), wo_v[:, :, :])
        nc.sync.dma_start(cm_sb[:, :], cm_d[:, :])
        nc.sync.dma_start(skr_sb[0:1, :], sk_d[:, :])
        nc.sync.dma_start(skr_sb[64:65, :], sk_d[:, :])
        nc.scalar.activation(esk_sb[0:1, :], skr_sb[0:1, :], AF.Exp)
        nc.scalar.activation(esk_sb[64:65, :], skr_sb[64:65, :], AF.Exp)
        # vp ones columns and zero filler ([1,1,0*63] pattern per pair region)
        vp_view = vp_sb[:].rearrange("p (k w) -> p k w", w=VP_W)
        vpc_view = vpc_d.rearrange("p (k w) -> p k w", w=65)
        nc.sync.dma_start(vp_view[:, :, 64:129], vpc_view[:, :, :])
        nc.sync.dma_start(vp_view[:, :, 257:322], vpc_view[:, :, :])
        nc.sync.dma_start(ind_sb[:, :], ind_d[:, :])

        def emit_proj_chunk(w_sb, t_sb, c):
            """Q^T/K^T chunk: both 128-row blocks for tokens
            [c*256,(c+1)*256) -> t_sb[:, mt*T + c*256 :+256], bf16."""
            ps = ps_s.tile([128, 1024], f32, tag="ps_s")
            for mt in range(2):
                for ci in range(NCC):
                    nc.tensor.matmul(
                        ps[:, mt * 512:mt * 512 + QB],
                        w_sb[:, ci * DH + mt * 128: ci * DH + (mt + 1) * 128],
                        xt_sb[:, ci * T + c * QB: ci * T + (c + 1) * QB],
                        start=(ci == 0), stop=(ci == NCC - 1))
            for mt in range(2):
                nc.vector.tensor_copy(
                    t_sb[:, mt * T + c * QB: mt * T + (c + 1) * QB],
                    ps[:, mt * 512:mt * 512 + QB])

        def emit_v_chunk(c):
            """V natural [t, d] for tokens [c*256,(c+1)*256) into the
            padded bf16 vp layout (kt tiles 2c, 2c+1)."""
            ps = ps_w.tile([128, 512], f32, tag="ps_w")
            for sub in range(2):
                tt = 2 * c + sub
                for ci in range(NCC):
                    nc.tensor.matmul(
                        ps[:, sub * DH:(sub + 1) * DH],
                        xt_sb[:, ci * T + tt * 128: ci * T + (tt + 1) * 128],
                        wv_sb[:, ci * DH: (ci + 1) * DH],
                        start=(ci == 0), stop=(ci == NCC - 1))
            for sub in range(2):
                tt = 2 * c + sub
                base = tt * VP_W
                s0 = sub * DH
                nc.vector.tensor_copy(vp_sb[:, base + 0: base + 64], ps[:, s0:s0 + 64])
                nc.vector.tensor_copy(vp_sb[:, base + 129: base + 257], ps[:, s0 + 64:s0 + 192])
                nc.vector.tensor_copy(vp_sb[:, base + 322: base + 386], ps[:, s0 + 192:s0 + 256])

        def pab_slot(p, qb):
            return ((2 * qb + p) % 3) * (NKT * 512)

        def emit_scores(p, qb):
            """All score groups for (pair p, q-block qb): row-tiled bf16
            kt pairs -> PSUM [128,1024] -> one exp -> pab bf16; mask the
            last (diagonal) group."""
            slot = pab_slot(p, qb)
            for g in range(qb + 1):
                ps = ps_s.tile([128, 1024], f32, tag="ps_s")
                for j in range(2):
                    kt = 2 * g + j
                    nc.tensor.matmul(
                        ps[:, j * 512: j * 512 + QB],
                        kt_sb[0:64, p * T + kt * 128: p * T + (kt + 1) * 128],
                        qt_sb[0:64, p * T + qb * QB: p * T + (qb + 1) * QB],
                        start=True, stop=True)
                    nc.tensor.matmul(
                        ps[:, j * 512 + QB: (j + 1) * 512],
                        kt_sb[64:128, p * T + kt * 128: p * T + (kt + 1) * 128],
                        qt_sb[64:128, p * T + qb * QB: p * T + (qb + 1) * QB],
                        start=True, stop=True)
                pg = pab_sb[:, slot + g * 1024: slot + (g + 1) * 1024]
                nc.scalar.activation(pg, ps[:, :], AF.Exp, scale=SCALE)
                if g == qb:
                    with nc.allow_low_precision(reason="0/1 mask mult"):
                        nc.vector.tensor_mul(pg, pg, cm_sb[:, :])

        def emit_pv(p, qb):
            """PV accumulation over all kt for (p, qb) -> psum oo
            [128, 512]: head A rows 0:64 + denom row 64 at cols 0:256,
            head B denom row 0 + rows 64:128 at cols 256:512."""
            slot = pab_slot(p, qb)
            oo = ps_o.tile([128, 512], f32, tag="ps_o")
            nkt = 2 * qb + 2
            for kt in range(nkt):
                base = kt * VP_W
                pk = slot + kt * 512
                nc.tensor.matmul(
                    oo[0:65, 0:QB],
                    vp_sb[:, base + VP_OFF[2 * p]: base + VP_OFF[2 * p] + 65],
                    pab_sb[:, pk: pk + QB],
                    start=(kt == 0), stop=(kt == nkt - 1))
                nc.tensor.matmul(
                    oo[:, QB:2 * QB],
                    vp_sb[:, base + VP_OFF[2 * p + 1]: base + VP_OFF[2 * p + 1] + 128],
                    pab_sb[:, pk + QB: pk + 2 * QB],
                    start=(kt == 0), stop=(kt == nkt - 1))
            return oo

        def emit_norm(p, qb, oo):
            """Softmax denominators (+sink) -> reciprocal -> PE broadcast
            -> normalized attn^T (bf16) into at_sb."""
            hA, hB = 2 * p, 2 * p + 1
            dn = row_p.tile([128, QB], f32, tag="row")
            rc = row_p.tile([128, QB], f32r, tag="rowr")
            nc.vector.tensor_scalar(
                out=dn[64:65, :], in0=oo[64:65, 0:QB],
                scalar1=esk_sb[64:65, hA:hA + 1], scalar2=None, op0=Alu.add)
            nc.vector.tensor_scalar(
                out=dn[0:1, :], in0=oo[0:1, QB:2 * QB],
                scalar1=esk_sb[0:1, hB:hB + 1], scalar2=None, op0=Alu.add)
            with nc.allow_low_precision(reason="f32r recip for PE broadcast"):
                nc.vector.reciprocal(rc[64:65, :], dn[64:65, :])
                nc.vector.reciprocal(rc[0:1, :], dn[0:1, :])
            bc = ps_w.tile([128, 512], f32, tag="ps_w")
            nc.tensor.matmul(bc[:, 0:QB], ind_sb[:, :], rc[:, :],
                             start=True, stop=True)
            bcs = row_p.tile([128, QB], f32, tag="bcs")
            nc.vector.tensor_copy(bcs[:, :], bc[:, 0:QB])
            cs = slice(p * T + qb * QB, p * T + (qb + 1) * QB)
            with nc.allow_low_precision(reason="bf16 attn out"):
                nc.vector.tensor_mul(at_sb[0:64, cs], oo[0:64, 0:QB],
                                     bcs[0:64, :])
                nc.vector.tensor_mul(at_sb[64:128, cs], oo[64:128, QB:2 * QB],
                                     bcs[64:128, :])

        def emit_wout(qc):
            """Output projection for q columns [qc*512,(qc+1)*512)."""
            for co in range(NCC):
                ps = ps_w.tile([128, 512], f32, tag="ps_w")
                for j in range(2):
                    nc.tensor.matmul(
                        ps[:, :],
                        wo_sb[:, j * C + co * 128: j * C + (co + 1) * 128],
                        at_sb[:, j * T + qc * 512: j * T + (qc + 1) * 512],
                        start=(j == 0), stop=(j == 1))
                yt = y_p.tile([128, 512], f32, tag="y")
                nc.vector.tensor_copy(yt[:, :], ps[:, :])
                nc.sync.dma_start(
                    yt_v[:, co:co + 1, qc * 512: (qc + 1) * 512],
                    yt[:, :].rearrange("p (n m) -> p n m", m=512))

        for _ in range(reps):
            # prolog
            emit_proj_chunk(wk_sb, kt_sb, 0)
            emit_proj_chunk(wq_sb, qt_sb, 0)
            emit_v_chunk(0)
            emit_scores(0, 0)
            emit_scores(1, 0)
            # steady steps: scores(p, s+1) runs one step ahead of PV(p, s);
            # pab ring-3 requires scores(1, s+1) to come after PV(0, s).
            for s in range(NQB):
                last = s == NQB - 1
                if not last:
                    emit_proj_chunk(wk_sb, kt_sb, s + 1)
                    emit_proj_chunk(wq_sb, qt_sb, s + 1)
                    emit_v_chunk(s + 1)
                    emit_scores(0, s + 1)
                oo = emit_pv(0, s)
                emit_norm(0, s, oo)
                if not last:
                    emit_scores(1, s + 1)
                oo = emit_pv(1, s)
                emit_norm(1, s, oo)
                if s % 2 == 1:
                    emit_wout(s // 2)

    nc.compile()
    return nc


def make_causal_masks():
    """cm [128, 1024] bf16 = [p1|p1|p2|p2]: p1 = (q >= k), p2 = (q >= k+128)
    for the diagonal kt-group layout [ktA:(A|B) | ktB:(A|B)]."""
    import ml_dtypes
    kl = np.arange(128)[:, None]
    ql = np.arange(QB)[None, :]
    p1 = (ql >= kl).astype(np.float32)
    p2 = (ql >= kl + 128).astype(np.float32)
    cm = np.concatenate([p1, p1, p2, p2], axis=1)
    return cm.astype(ml_dtypes.bfloat16)


def shard_inputs(x, W_Q, W_K, W_V, W_out, sink):
    import ml_dtypes
    cm = make_causal_masks()
    vpc = np.zeros((128, 65), dtype=np.float32)
    vpc[:, 0:2] = 1.0
    vpc = np.tile(vpc, (1, NKT)).astype(ml_dtypes.bfloat16)
    ind = np.zeros((128, 128), dtype=np.float32)
    ind[64, 0:64] = 1.0   # head A recip (row 64) -> rows 0-63
    ind[0, 64:128] = 1.0  # head B recip (row 0) -> rows 64-127
    in_maps = []
    for c in range(NCORES):
        b, g = divmod(c, G)
        cols = slice(g * DH, (g + 1) * DH)
        in_maps.append({
            "xt": np.ascontiguousarray(x[b].T),
            "wq": np.ascontiguousarray(W_Q[:, cols]),
            "wk": np.ascontiguousarray(W_K[:, cols]),
            "wv": np.ascontiguousarray(W_V[:, cols]),
            "wo": np.ascontiguousarray(W_out[cols, :]).astype(ml_dtypes.bfloat16),
            "sk": np.ascontiguousarray(sink[g * G:(g + 1) * G][None, :]),
            "cm": cm,
            "vpc": vpc,
            "ind": ind,
        })
    return in_maps


def gather_outputs(results):
    out = np.zeros((B, T, C), dtype=np.float32)
    for b in range(B):
        acc = np.zeros((C, T), dtype=np.float32)
        for g in range(G):
            acc += results[b * G + g]["yt"]
        out[b] = acc.T
    return out


_CACHE = {}


def _get_program():
    if "nc" not in _CACHE:
        _CACHE["nc"] = build_program(reps=1)
    return _CACHE["nc"]


def kernel(x, W_Q, W_K, W_V, W_out, sink):
    from concourse.bass_utils import run_bass_kernel_spmd

    x = np.asarray(x, dtype=np.float32)
    W_Q = np.asarray(W_Q, dtype=np.float32)
    W_K = np.asarray(W_K, dtype=np.float32)
    W_V = np.asarray(W_V, dtype=np.float32)
    W_out = np.asarray(W_out, dtype=np.float32)
    sink = np.asarray(sink, dtype=np.float32)

    nc = _get_program()
    in_maps = shard_inputs(x, W_Q, W_K, W_V, W_out, sink)
    res = run_bass_kernel_spmd(nc, in_maps, core_ids=list(range(NCORES)))
    return gather_outputs(res.results)


# revision 18
# speedup vs baseline: 1.6997x; 1.6997x over previous
"""Trainium2 Bass kernel: 16-head causal attention with sink logit.

Contract: kernel(**inputs) takes the FULL inputs of the reference
(x [2,2048,1024], W_Q/W_K/W_V/W_out [1024,1024], sink [16]) and returns
the FULL output [2,2048,1024], running on 8 NeuronCores.

Sharding: core c = b*4 + g handles batch b and heads [4g, 4g+4).
Each core computes yT_partial [1024, 2048] = W_out_slice^T @ attn^T;
host sums the 4 partials per batch and transposes.

v2 design (vs baseline):
- scores in bf16: FWL weight loads + 2-way row-tiled (K=64) head pairs
  run concurrently on row-groups 0/64 of the PE array.
- PV in bf16 with the ones-row denominator trick (V-stationary, output
  is attn^T directly, M=65/128 padded lhsT).
- exp batched [128,1024] straight from PSUM on ScalarE.
- q-blocks of 256 (less masked-region waste); f32r projections chopped
  into 256-token chunks woven between attention steps so the PE always
  has work while ScalarE runs the exp stream.
"""

import sys
import numpy as np

if "/opt/trn_rl_repo" not in sys.path:
    sys.path.insert(0, "/opt/trn_rl_repo")

B, T, C = 2, 2048, 1024
H, D = 16, 64
G = 4                # heads per core
DH = G * D           # 256 head-dims per core
NCORES = 8
QB = 256             # q block (attention step granularity)
NQB = T // QB        # 8
NKT = T // 128       # 16 k-tiles
NCC = C // 128       # 8 contraction chunks over C
SCALE = 1.0 / float(np.sqrt(D))

# vp per-kt slot layout (386 cols per kt), bf16:
#   pair p head A: [V(64) | one]             off p*193+0,  width 65,  denom row 64
#   pair p head B: [one | zeros(63) | V(64)] off p*193+65, width 128, denom row 0
VP_W = 386
VP_OFF = [0, 65, 193, 258]


def build_program(reps=1):
    """Build the per-core Bass program. reps>1 repeats the compute body
    (same inputs -> same outputs) for differential wall-clock timing."""
    from contextlib import ExitStack

    import concourse.bass as bass
    import concourse.tile as tile
    from concourse import bacc, mybir

    f32 = mybir.dt.float32
    f32r = mybir.dt.float32r
    bf16 = mybir.dt.bfloat16
    AF = mybir.ActivationFunctionType
    Alu = mybir.AluOpType

    nc = bacc.Bacc("TRN2", target_bir_lowering=False, debug=False)

    xt_d = nc.dram_tensor("xt", [C, T], f32r, kind="ExternalInput").ap()
    wq_d = nc.dram_tensor("wq", [C, DH], f32r, kind="ExternalInput").ap()
    wk_d = nc.dram_tensor("wk", [C, DH], f32r, kind="ExternalInput").ap()
    wv_d = nc.dram_tensor("wv", [C, DH], f32r, kind="ExternalInput").ap()
    wo_d = nc.dram_tensor("wo", [DH, C], bf16, kind="ExternalInput").ap()
    sk_d = nc.dram_tensor("sk", [1, G], f32, kind="ExternalInput").ap()
    cm_d = nc.dram_tensor("cm", [128, 1024], bf16, kind="ExternalInput").ap()
    vpc_d = nc.dram_tensor("vpc", [128, NKT * 65], bf16, kind="ExternalInput").ap()
    ind_d = nc.dram_tensor("ind", [128, 128], f32r, kind="ExternalInput").ap()
    zr_d = nc.dram_tensor("zr", [128, QB], f32r, kind="ExternalInput").ap()
    yt_d = nc.dram_tensor("yt", [C, T], f32, kind="ExternalOutput").ap()

    xt_v = xt_d.rearrange("(n p) m -> p n m", p=128)   # [128, 8, 2048]
    wq_v = wq_d.rearrange("(n p) m -> p n m", p=128)   # [128, 8, 256]
    wk_v = wk_d.rearrange("(n p) m -> p n m", p=128)
    wv_v = wv_d.rearrange("(n p) m -> p n m", p=128)
    wo_v = wo_d.rearrange("(n p) m -> p n m", p=128)   # [128, 2, 1024]
    yt_v = yt_d.rearrange("(n p) m -> p n m", p=128)   # [128, 8, 2048]

    with tile.TileContext(nc) as tc, ExitStack() as ctx:
        P = lambda name, bufs: ctx.enter_context(tc.tile_pool(name=name, bufs=bufs))
        const_p = P("const", 1)
        big_p = P("big", 1)
        y_p = P("y", 2)
        row_p = P("row", 2)
        # PSUM: ps_s 2x2 banks + ps_o 2x1 + ps_w 2x1 = 8 banks
        ps_s = ctx.enter_context(tc.tile_pool(name="ps_s", bufs=2, space="PSUM"))
        ps_o = ctx.enter_context(tc.tile_pool(name="ps_o", bufs=2, space="PSUM"))
        ps_w = ctx.enter_context(tc.tile_pool(name="ps_w", bufs=2, space="PSUM"))

        # ---- persistent SBUF tensors ----
        xt_sb = big_p.tile([128, NCC * T], f32r, tag="xt")           # 64KB/part
        wq_sb = big_p.tile([128, NCC * DH], f32r, tag="wq")
        wk_sb = big_p.tile([128, NCC * DH], f32r, tag="wk")
        wv_sb = big_p.tile([128, NCC * DH], f32r, tag="wv")
        wo_sb = big_p.tile([128, 2 * C], bf16, tag="wo")
        qt_sb = big_p.tile([128, 2 * T], bf16, tag="qt")
        kt_sb = big_p.tile([128, 2 * T], bf16, tag="kt")
        vp_sb = big_p.tile([128, NKT * VP_W], bf16, tag="vp")
        pab_sb = big_p.tile([128, 3 * NKT * 512], bf16, tag="pab")   # ring of 3
        at_sb = big_p.tile([128, 2 * T], bf16, tag="at")             # attn^T normalized
        rc_sb = big_p.tile([128, QB], f32r, tag="rc")
        cm_sb = const_p.tile([128, 1024], bf16, tag="cm")
        ind_sb = const_p.tile([128, 128], f32r, tag="ind")
        skr_sb = const_p.tile([128, G], f32, tag="skr")
        esk_sb = const_p.tile([128, G], f32, tag="esk")

        # ---- one-time loads + constants (outside reps) ----
        for i in range(NCC):
            nc.sync.dma_start(xt_sb[:, i * T:(i + 1) * T], xt_v[:, i, :])
        nc.sync.dma_start(
            wq_sb[:].rearrange("p (n m) -> p n m", m=DH), wq_v[:, :, :])
        nc.sync.dma_start(
            wk_sb[:].rearrange("p (n m) -> p n m", m=DH), wk_v[:, :, :])
        nc.sync.dma_start(
            wv_sb[:].rearrange("p (n m) -> p n m", m=DH), wv_v[:, :, :])
        nc.sync.dma_start(
            wo_sb[:].rearrange("p (n m) -> p n m", m=C), wo_v[:, :, :])
        nc.sync.dma_start(cm_sb[:, :], cm_d[:, :])
        nc.sync.dma_start(skr_sb[0:1, :], sk_d[:, :])
        nc.sync.dma_start(skr_sb[64:65, :], sk_d[:, :])
        nc.scalar.activation(esk_sb[0:1, :], skr_sb[0:1, :], AF.Exp)
        nc.scalar.activation(esk_sb[64:65, :], skr_sb[64:65, :], AF.Exp)
        # vp ones columns and zero filler ([1,1,0*63] pattern per pair region)
        vp_view = vp_sb[:].rearrange("p (k w) -> p k w", w=VP_W)
        vpc_view = vpc_d.rearrange("p (k w) -> p k w", w=65)
        nc.sync.dma_start(vp_view[:, :, 64:129], vpc_view[:, :, :])
        nc.sync.dma_start(vp_view[:, :, 257:322], vpc_view[:, :, :])
        nc.sync.dma_start(ind_sb[:, :], ind_d[:, :])
        nc.sync.dma_start(rc_sb[:, :], zr_d[:, :])

        def emit_proj_chunk(w_sb, t_sb, c):
            """Q^T/K^T chunk: both 128-row blocks for tokens
            [c*256,(c+1)*256) -> t_sb[:, mt*T + c*256 :+256], bf16."""
            ps = ps_s.tile([128, 1024], f32, tag="ps_s")
            for mt in range(2):
                for ci in range(NCC):
                    nc.tensor.matmul(
                        ps[:, mt * 512:mt * 512 + QB],
                        w_sb[:, ci * DH + mt * 128: ci * DH + (mt + 1) * 128],
                        xt_sb[:, ci * T + c * QB: ci * T + (c + 1) * QB],
                        start=(ci == 0), stop=(ci == NCC - 1))
            for mt in range(2):
                nc.vector.tensor_copy(
                    t_sb[:, mt * T + c * QB: mt * T + (c + 1) * QB],
                    ps[:, mt * 512:mt * 512 + QB])

        def emit_v_chunk(c):
            """V natural [t, d] for tokens [c*256,(c+1)*256) into the
            padded bf16 vp layout (kt tiles 2c, 2c+1)."""
            ps = ps_w.tile([128, 512], f32, tag="ps_w")
            for sub in range(2):
                tt = 2 * c + sub
                for ci in range(NCC):
                    # one accumulation group for the whole bank: start
                    # clears has_written bank-wide, so only the first MM
                    # starts and only the last stops
                    nc.tensor.matmul(
                        ps[:, sub * DH:(sub + 1) * DH],
                        xt_sb[:, ci * T + tt * 128: ci * T + (tt + 1) * 128],
                        wv_sb[:, ci * DH: (ci + 1) * DH],
                        start=(sub == 0 and ci == 0),
                        stop=(sub == 1 and ci == NCC - 1))
            for sub in range(2):
                tt = 2 * c + sub
                base = tt * VP_W
                s0 = sub * DH
                nc.vector.tensor_copy(vp_sb[:, base + 0: base + 64], ps[:, s0:s0 + 64])
                nc.vector.tensor_copy(vp_sb[:, base + 129: base + 257], ps[:, s0 + 64:s0 + 192])
                nc.vector.tensor_copy(vp_sb[:, base + 322: base + 386], ps[:, s0 + 192:s0 + 256])

        def pab_slot(p, qb):
            return ((2 * qb + p) % 3) * (NKT * 512)

        def emit_scores(p, qb):
            """All score groups for (pair p, q-block qb): row-tiled bf16
            kt pairs -> PSUM [128,1024] -> one exp -> pab bf16; mask the
            last (diagonal) group."""
            slot = pab_slot(p, qb)
            for g in range(qb + 1):
                ps = ps_s.tile([128, 1024], f32, tag="ps_s")
                for j in range(2):
                    kt = 2 * g + j
                    # row-tiled concurrent pair: A drains to bank 0
                    # (cols 0:512), B to bank 1 (cols 512:1024) so the
                    # concurrent MMs never share a PSUM bank
                    nc.tensor.matmul(
                        ps[:, j * QB: (j + 1) * QB],
                        kt_sb[0:64, p * T + kt * 128: p * T + (kt + 1) * 128],
                        qt_sb[0:64, p * T + qb * QB: p * T + (qb + 1) * QB],
                        start=(j == 0), stop=(j == 1))
                    nc.tensor.matmul(
                        ps[:, 512 + j * QB: 512 + (j + 1) * QB],
                        kt_sb[64:128, p * T + kt * 128: p * T + (kt + 1) * 128],
                        qt_sb[64:128, p * T + qb * QB: p * T + (qb + 1) * QB],
                        start=(j == 0), stop=(j == 1))
                pg = pab_sb[:, slot + g * 1024: slot + (g + 1) * 1024]
                nc.scalar.activation(pg, ps[:, :], AF.Exp, scale=SCALE)
                if g == qb:
                    with nc.allow_low_precision(reason="0/1 mask mult"):
                        nc.vector.tensor_mul(pg, pg, cm_sb[:, :])

        def emit_pv(p, qb):
            """PV accumulation over all kt for (p, qb) -> psum oo
            [128, 512]: head A rows 0:64 + denom row 64 at cols 0:256,
            head B denom row 0 + rows 64:128 at cols 256:512."""
            slot = pab_slot(p, qb)
            oo = ps_o.tile([128, 512], f32, tag="ps_o")
            nkt = 2 * qb + 2
            for kt in range(nkt):
                base = kt * VP_W
                # pab group layout: [A_kt(2g) | A_kt(2g+1) | B_kt(2g) | B_kt(2g+1)]
                g, j = divmod(kt, 2)
                pkA = slot + g * 1024 + j * QB
                pkB = pkA + 512
                # A and B accumulate in the same bank: single group. The
                # group-starting MM must cover all written partitions, so
                # B (M=128) goes first; A (M=65) is a partition subset.
                # skip_group_check: A/B regions are disjoint columns of one
                # bank; per-region last-writer deps + DVE FIFO give correct
                # ordering on HW, but the sim's dep-driven replay would flag
                # reads of the B region before the group-closing A matmul.
                nc.tensor.matmul(
                    oo[:, QB:2 * QB],
                    vp_sb[:, base + VP_OFF[2 * p + 1]: base + VP_OFF[2 * p + 1] + 128],
                    pab_sb[:, pkB: pkB + QB],
                    start=(kt == 0), stop=False, skip_group_check=True)
                nc.tensor.matmul(
                    oo[0:65, 0:QB],
                    vp_sb[:, base + VP_OFF[2 * p]: base + VP_OFF[2 * p] + 65],
                    pab_sb[:, pkA: pkA + QB],
                    start=False, stop=(kt == nkt - 1), skip_group_check=True)
            return oo

        def emit_norm(p, qb, oo):
            """Softmax denominators (+sink) -> reciprocal -> PE broadcast
            -> normalized attn^T (bf16) into at_sb."""
            hA, hB = 2 * p, 2 * p + 1
            dn = row_p.tile([128, QB], f32, tag="row")
            nc.vector.tensor_scalar(
                out=dn[64:65, :], in0=oo[64:65, 0:QB],
                scalar1=esk_sb[64:65, hA:hA + 1], scalar2=None, op0=Alu.add)
            nc.vector.tensor_scalar(
                out=dn[0:1, :], in0=oo[0:1, QB:2 * QB],
                scalar1=esk_sb[0:1, hB:hB + 1], scalar2=None, op0=Alu.add)
            with nc.allow_low_precision(reason="f32r recip for PE broadcast"):
                nc.vector.reciprocal(rc_sb[64:65, :], dn[64:65, :])
                nc.vector.reciprocal(rc_sb[0:1, :], dn[0:1, :])
            bc = ps_w.tile([128, 512], f32, tag="ps_w")
            nc.tensor.matmul(bc[:, 0:QB], ind_sb[:, :], rc_sb[:, :],
                             start=True, stop=True)
            bcs = row_p.tile([128, QB], f32, tag="bcs")
            nc.vector.tensor_copy(bcs[:, :], bc[:, 0:QB])
            cs = slice(p * T + qb * QB, p * T + (qb + 1) * QB)
            with nc.allow_low_precision(reason="bf16 attn out"):
                nc.vector.tensor_mul(at_sb[0:64, cs], oo[0:64, 0:QB],
                                     bcs[0:64, :])
                nc.vector.tensor_mul(at_sb[64:128, cs], oo[64:128, QB:2 * QB],
                                     bcs[64:128, :])

        def emit_wout(qc):
            """Output projection for q columns [qc*512,(qc+1)*512)."""
            for co in range(NCC):
                ps = ps_w.tile([128, 512], f32, tag="ps_w")
                for j in range(2):
                    nc.tensor.matmul(
                        ps[:, :],
                        wo_sb[:, j * C + co * 128: j * C + (co + 1) * 128],
                        at_sb[:, j * T + qc * 512: j * T + (qc + 1) * 512],
                        start=(j == 0), stop=(j == 1))
                yt = y_p.tile([128, 512], f32, tag="y")
                nc.vector.tensor_copy(yt[:, :], ps[:, :])
                nc.sync.dma_start(
                    yt_v[:, co:co + 1, qc * 512: (qc + 1) * 512],
                    yt[:, :].rearrange("p (n m) -> p n m", m=512))

        for _ in range(reps):
            # prolog
            emit_proj_chunk(wk_sb, kt_sb, 0)
            emit_proj_chunk(wq_sb, qt_sb, 0)
            emit_v_chunk(0)
            emit_scores(0, 0)
            emit_scores(1, 0)
            # steady steps: scores(p, s+1) runs one step ahead of PV(p, s);
            # pab ring-3 requires scores(1, s+1) to come after PV(0, s).
            for s in range(NQB):
                last = s == NQB - 1
                if not last:
                    emit_proj_chunk(wk_sb, kt_sb, s + 1)
                    emit_proj_chunk(wq_sb, qt_sb, s + 1)
                    emit_v_chunk(s + 1)
                    emit_scores(0, s + 1)
                oo = emit_pv(0, s)
                emit_norm(0, s, oo)
                if not last:
                    emit_scores(1, s + 1)
                oo = emit_pv(1, s)
                emit_norm(1, s, oo)
                if s % 2 == 1:
                    emit_wout(s // 2)

    nc.compile()
    return nc


def make_causal_masks():
    """cm [128, 1024] bf16 = [p1|p2|p1|p2]: p1 = (q >= k), p2 = (q >= k+128)
    for the diagonal kt-group layout [A_kt0|A_kt1|B_kt0|B_kt1]."""
    import ml_dtypes
    kl = np.arange(128)[:, None]
    ql = np.arange(QB)[None, :]
    p1 = (ql >= kl).astype(np.float32)
    p2 = (ql >= kl + 128).astype(np.float32)
    cm = np.concatenate([p1, p2, p1, p2], axis=1)
    return cm.astype(ml_dtypes.bfloat16)


def shard_inputs(x, W_Q, W_K, W_V, W_out, sink):
    import ml_dtypes
    cm = make_causal_masks()
    vpc = np.zeros((128, 65), dtype=np.float32)
    vpc[:, 0:2] = 1.0
    vpc = np.tile(vpc, (1, NKT)).astype(ml_dtypes.bfloat16)
    ind = np.zeros((128, 128), dtype=np.float32)
    ind[64, 0:64] = 1.0   # head A recip (row 64) -> rows 0-63
    ind[0, 64:128] = 1.0  # head B recip (row 0) -> rows 64-127
    in_maps = []
    for c in range(NCORES):
        b, g = divmod(c, G)
        cols = slice(g * DH, (g + 1) * DH)
        in_maps.append({
            "xt": np.ascontiguousarray(x[b].T),
            "wq": np.ascontiguousarray(W_Q[:, cols]),
            "wk": np.ascontiguousarray(W_K[:, cols]),
            "wv": np.ascontiguousarray(W_V[:, cols]),
            "wo": np.ascontiguousarray(W_out[cols, :]).astype(ml_dtypes.bfloat16),
            "sk": np.ascontiguousarray(sink[g * G:(g + 1) * G][None, :]),
            "cm": cm,
            "vpc": vpc,
            "ind": ind,
            "zr": np.zeros((128, QB), dtype=np.float32),
        })
    return in_maps


def gather_outputs(results):
    out = np.zeros((B, T, C), dtype=np.float32)
    for b in range(B):
        acc = np.zeros((C, T), dtype=np.float32)
        for g in range(G):
            acc += results[b * G + g]["yt"]
        out[b] = acc.T
    return out


_CACHE = {}


def _get_program():
    if "nc" not in _CACHE:
        _CACHE["nc"] = build_program(reps=1)
    return _CACHE["nc"]


def kernel(x, W_Q, W_K, W_V, W_out, sink):
    from concourse.bass_utils import run_bass_kernel_spmd

    x = np.asarray(x, dtype=np.float32)
    W_Q = np.asarray(W_Q, dtype=np.float32)
    W_K = np.asarray(W_K, dtype=np.float32)
    W_V = np.asarray(W_V, dtype=np.float32)
    W_out = np.asarray(W_out, dtype=np.float32)
    sink = np.asarray(sink, dtype=np.float32)

    nc = _get_program()
    in_maps = shard_inputs(x, W_Q, W_K, W_V, W_out, sink)
    res = run_bass_kernel_spmd(nc, in_maps, core_ids=list(range(NCORES)))
    return gather_outputs(res.results)
